# revision 6
# baseline (speedup 1.0000x reference)
"""DendriticMLP Trainium2 kernel — 8-core batch-data-parallel, exact fp32.

Architecture (per core, B_local=512 rows):
  y1 = x @ w1.T + b1                (PE, fp32 exact)
  dend1 = ctx @ seg1_flat.T         (PE) -> per-(b,h) max/min over 10 segments (DVE
          strided reduce from PSUM) -> sel = absmax-signed value via
          where(max+min>0, max, min) -> gate = sigmoid(sel) (ACT)
  g = y1 * gate; top-k (k=102) per row via threshold bisection on
          count(g >= t) (DVE tensor_scalar + accum), 23 iterations;
          h = (g >= lo) * g
  h transposed 128x128 blocks on PE for the next layer's stationary operand.
  layer 2 same; out = h2 @ w_out.T + b_out.

All matmuls native fp32 (hi/lo 2-pass in HW, exact); top-k threshold
bisection converges to kth-largest exactly (see where() analysis).
"""
import numpy as np
from contextlib import ExitStack

import concourse.bass as bass
import concourse.tile as tile
from concourse import bacc, mybir, masks
from concourse.bass_utils import run_bass_kernel_spmd

F32 = mybir.dt.float32
AF = mybir.ActivationFunctionType
OP = mybir.AluOpType
AX = mybir.AxisListType

# problem dims (hardcoded per contract)
B, D_IN, H, S, D_CTX, D_OUT = 4096, 1024, 2048, 10, 1024, 1024
KW = 102                 # k-winners per row
NCORES = 8
BL = B // NCORES         # 512 rows per core
BT = BL // 128           # 4 b-tiles of 128 rows
NITER = 23               # bisection iterations

HS = H * S               # 20480
CHW = 510                # dend chunk width (51 groups of 10)
NCH = HS // CHW          # 40 full chunks
TAIL = HS - NCH * CHW    # 80 (8 groups)
GR = CHW // S            # 51 groups per chunk
KT_IN = D_IN // 128      # 8 contraction tiles for d=1024
KT_H = H // 128          # 16 contraction tiles for d=2048


def build_kernel():
    nc = bacc.Bacc("TRN2", target_bir_lowering=False, debug=False,
                   num_devices=NCORES)

    def din(name, shape):
        return nc.dram_tensor(name, shape, F32, kind="ExternalInput").ap()

    xT = din("xT", [D_IN, BL])
    ctxT = din("ctxT", [D_IN, BL])
    w1t = din("w1t", [H // 512, KT_IN, 128, 512])
    w2t = din("w2t", [H // 512, KT_H, 128, 512])
    wot = din("wot", [D_OUT // 512, KT_H, 128, 512])
    sg1a = din("sg1a", [NCH, KT_IN, 128, CHW])
    sg1b = din("sg1b", [KT_IN, 128, TAIL])
    sg2a = din("sg2a", [NCH, KT_IN, 128, CHW])
    sg2b = din("sg2b", [KT_IN, 128, TAIL])
    b1d = din("b1d", [1, H])
    b2d = din("b2d", [1, H])
    bod = din("bod", [1, D_OUT])
    out_d = nc.dram_tensor("out", [BL, D_OUT], F32, kind="ExternalOutput").ap()

    with tile.TileContext(nc) as tc, ExitStack() as ctx:
        cpool = ctx.enter_context(tc.tile_pool(name="const", bufs=1))
        apool = ctx.enter_context(tc.tile_pool(name="acts", bufs=1))
        ypool = ctx.enter_context(tc.tile_pool(name="y", bufs=BT))
        selpool = ctx.enter_context(tc.tile_pool(name="sel", bufs=BT))
        mnpool = ctx.enter_context(tc.tile_pool(name="mn", bufs=BT))
        htpool = ctx.enter_context(tc.tile_pool(name="ht", bufs=1))
        wpool = ctx.enter_context(tc.tile_pool(name="w", bufs=3))
        spool = ctx.enter_context(tc.tile_pool(name="seg", bufs=8))
        outpool = ctx.enter_context(tc.tile_pool(name="osb", bufs=2))
        tinypool = ctx.enter_context(tc.tile_pool(name="tiny", bufs=1))
        psy = ctx.enter_context(tc.tile_pool(name="psy", bufs=BT, space="PSUM"))
        psd = ctx.enter_context(tc.tile_pool(name="psd", bufs=3, space="PSUM"))
        pst = ctx.enter_context(tc.tile_pool(name="pst", bufs=1, space="PSUM"))

        # constants
        identity = cpool.tile([128, 128], F32)
        masks.make_identity(nc, identity[:])
        ones = cpool.tile([1, 128], F32)
        nc.gpsimd.memset(ones[:], 1.0)
        b1sb = cpool.tile([1, H], F32)
        nc.sync.dma_start(b1sb[:], b1d)
        b2sb = cpool.tile([1, H], F32)
        nc.sync.dma_start(b2sb[:], b2d)
        bosb = cpool.tile([1, D_OUT], F32)
        nc.sync.dma_start(bosb[:], bod)

        # activations stationary: [128, kk*BL + bt*128] layout
        xT_sb = apool.tile([128, KT_IN * BL], F32, tag="xt_scr")
        ctxT_sb = apool.tile([128, KT_IN * BL], F32)
        for kk in range(KT_IN):
            nc.sync.dma_start(xT_sb[:, kk * BL:(kk + 1) * BL],
                              xT[kk * 128:(kk + 1) * 128, :])
            nc.sync.dma_start(ctxT_sb[:, kk * BL:(kk + 1) * BL],
                              ctxT[kk * 128:(kk + 1) * 128, :])

        def st_ap(sb, kk, bt):
            return sb[:, kk * BL + bt * 128: kk * BL + (bt + 1) * 128]

        # per-bt tiny state: cols 0=M 1=lo 2=w 3=t 4=pred 5=cnt
        tiny = [tinypool.tile([128, 8], F32, tag=f"tiny{bt}", name=f"tiny{bt}") for bt in range(BT)]

        h1T = htpool.tile([128, KT_H * 512], F32)   # reused for h2T
        scr_cell = []

        def get_scr():
            if not scr_cell:
                scr_cell.append(apool.tile([128, H], F32, tag="xt_scr",
                                           name="scr"))
            return scr_cell[0]

        def yphase(stat_sb_or_ht, kt, wt_dram, bias_sb, width, lay):
            """Dense y = act @ W.T (+bias). Returns list of BT y tiles [128, width]."""
            ytiles = [ypool.tile([128, H], F32, tag="y", name=f"y{lay}_{i}") for i in range(BT)]
            nch = width // 512
            for n in range(nch):
                ps = [psy.tile([128, 512], F32, tag="psy", name=f"psy{i}") for i in range(BT)]
                for k in range(kt):
                    wt = wpool.tile([128, 512], F32, tag="w")
                    nc.sync.dma_start(wt[:], wt_dram[n, k])
                    for bt in range(BT):
                        if lay == 0:
                            lhsT = st_ap(xT_sb, k, bt)
                        else:
                            lhsT = stat_sb_or_ht[:, k * 512 + bt * 128:
                                                 k * 512 + (bt + 1) * 128]
                        nc.tensor.matmul(ps[bt][:], lhsT, wt[:],
                                         start=(k == 0), stop=False)
                for bt in range(BT):
                    nc.tensor.matmul(ps[bt][:], ones[:],
                                     bias_sb[:, n * 512:(n + 1) * 512],
                                     start=False, stop=True)
                for bt in range(BT):
                    nc.scalar.activation(ytiles[bt][:, n * 512:(n + 1) * 512],
                                         ps[bt][:], AF.Copy)
            return ytiles

        def dendphase(sga, sgb, seltiles, mntiles):
            """dend matmuls + segment max/min reduces into sel (max) and mn."""
            for c in range(NCH + 1):
                w = CHW if c < NCH else TAIL
                g = GR if c < NCH else TAIL // S
                segs = []
                for k in range(KT_IN):
                    stile = spool.tile([128, CHW], F32, tag="seg")
                    if c < NCH:
                        nc.sync.dma_start(stile[:, :w], sga[c, k])
                    else:
                        nc.sync.dma_start(stile[:, :w], sgb[k])
                    segs.append(stile)
                for bt in range(BT):
                    pd = psd.tile([128, CHW], F32, tag="psd")
                    for k in range(KT_IN):
                        nc.tensor.matmul(pd[:, :w], st_ap(ctxT_sb, k, bt),
                                         segs[k][:, :w],
                                         start=(k == 0), stop=(k == KT_IN - 1))
                    view = pd[:, :w].rearrange("p (g s) -> p g s", s=S)
                    nc.vector.tensor_reduce(
                        seltiles[bt][:, c * GR:c * GR + g], view,
                        axis=AX.X, op=OP.max)
                    nc.vector.tensor_reduce(
                        mntiles[bt][:, c * GR:c * GR + g], view,
                        axis=AX.X, op=OP.min)

        def selgate(seltiles, mntiles):
            """sel=where(mx+mn>0,mx,mn) in-place over mx, then sigmoid -> gate."""
            for bt in range(BT):
                mx = seltiles[bt][:]
                mn = mntiles[bt][:]
                scr = get_scr()
                nc.vector.tensor_add(scr[:], mx, mn)
                nc.vector.tensor_scalar(scr[:], scr[:], 0.0, None, op0=OP.is_le)
                nc.vector.copy_predicated(mx, scr[:].bitcast(mybir.dt.int32), mn)
                nc.scalar.activation(mx, mx, AF.Sigmoid)

        def gate_mul(ytiles, seltiles):
            """g = y*gate in place on y tile; M = absmax(g) into tiny col 0."""
            for bt in range(BT):
                nc.vector.tensor_mul(ytiles[bt][:], ytiles[bt][:],
                                     seltiles[bt][:])
                nc.vector.tensor_reduce(tiny[bt][:, 0:1], ytiles[bt][:],
                                        axis=AX.X, op=OP.max,
                                        apply_absolute_value=True)

        def bisect_mask(ytiles):
            """top-k threshold per row via bisection; mask y in place."""
            for bt in range(BT):
                t = tiny[bt]
                M, lo, w, tt_, pred, cnt = (t[:, i:i + 1] for i in range(6))
                nc.vector.tensor_scalar(tt_, M, 1.001, 1e-30, op0=OP.mult,
                                        op1=OP.add)
                nc.vector.tensor_scalar_mul(lo, tt_, -1.0)
                nc.vector.tensor_scalar_mul(w, tt_, 2.0)
            for it in range(NITER):
                for bt in range(BT):
                    t = tiny[bt]
                    M, lo, w, tt_, pred, cnt = (t[:, i:i + 1] for i in range(6))
                    nc.vector.tensor_scalar_mul(w, w, 0.5)
                    nc.vector.tensor_add(tt_, lo, w)
                    nc.vector.scalar_tensor_tensor(
                        get_scr()[:], ytiles[bt][:], tt_, ytiles[bt][:],
                        op0=OP.is_ge, op1=OP.bypass, accum_out=cnt)
                    nc.vector.tensor_scalar(pred, cnt, float(KW), None,
                                            op0=OP.is_ge)
                    nc.vector.scalar_tensor_tensor(lo, pred, w, lo,
                                                   op0=OP.mult, op1=OP.add)
            for bt in range(BT):
                lo = tiny[bt][:, 1:2]
                nc.vector.scalar_tensor_tensor(ytiles[bt][:], ytiles[bt][:],
                                               lo, ytiles[bt][:],
                                               op0=OP.is_ge, op1=OP.mult)

        def transpose_to(ytiles, dst):
            for bt in range(BT):
                for kb in range(KT_H):
                    pt = pst.tile([128, 128], F32, tag="pst")
                    nc.tensor.transpose(pt[:],
                                        ytiles[bt][:, kb * 128:(kb + 1) * 128],
                                        identity[:])
                    nc.scalar.activation(
                        dst[:, kb * 512 + bt * 128: kb * 512 + (bt + 1) * 128],
                        pt[:], AF.Copy)

        # ---------------- layer 1 ----------------
        sel1 = [selpool.tile([128, H], F32, tag="sel", name=f"sel1_{i}") for i in range(BT)]
        mn1 = [mnpool.tile([128, H], F32, tag="mn", name=f"mn1_{i}") for i in range(BT)]
        y1 = yphase(None, KT_IN, w1t, b1sb, H, lay=0)
        dendphase(sg1a, sg1b, sel1, mn1)
        selgate(sel1, mn1)
        gate_mul(y1, sel1)
        bisect_mask(y1)

        # dend2 early (keeps PE busy during layer-1 bisection)
        sel2 = [selpool.tile([128, H], F32, tag="sel", name=f"sel2_{i}") for i in range(BT)]
        mn2 = [mnpool.tile([128, H], F32, tag="mn", name=f"mn2_{i}") for i in range(BT)]
        dendphase(sg2a, sg2b, sel2, mn2)
        selgate(sel2, mn2)

        transpose_to(y1, h1T)

        # ---------------- layer 2 ----------------
        y2 = yphase(h1T[:], KT_H, w2t, b2sb, H, lay=1)
        gate_mul(y2, sel2)
        bisect_mask(y2)
        transpose_to(y2, h1T)  # h2T reuses the same arena

        # ---------------- output layer ----------------
        for n in range(D_OUT // 512):
            ps = [psy.tile([128, 512], F32, tag="psy", name=f"psy{i}") for i in range(BT)]
            for k in range(KT_H):
                wt = wpool.tile([128, 512], F32, tag="w")
                nc.sync.dma_start(wt[:], wot[n, k])
                for bt in range(BT):
                    lhsT = h1T[:, k * 512 + bt * 128: k * 512 + (bt + 1) * 128]
                    nc.tensor.matmul(ps[bt][:], lhsT, wt[:],
                                     start=(k == 0), stop=False)
            for bt in range(BT):
                nc.tensor.matmul(ps[bt][:], ones[:],
                                 bosb[:, n * 512:(n + 1) * 512],
                                 start=False, stop=True)
            for bt in range(BT):
                osb = outpool.tile([128, 512], F32, tag="osb")
                nc.scalar.activation(osb[:], ps[bt][:], AF.Copy)
                nc.sync.dma_start(
                    out_d[bt * 128:(bt + 1) * 128, n * 512:(n + 1) * 512],
                    osb[:])

    nc.compile()
    return nc


def _prep_inputs(x, context, w1, b1, seg1, w2, b2, seg2, w_out, b_out):
    """Host-side reshapes into the DMA-friendly tiled layouts."""
    c = np.ascontiguousarray

    def tile_wt(w, kt, nch):
        # w [out, in] -> wT [in, out] -> [nch, kt, 128, 512]
        wT = w.T
        return c(wT.reshape(kt, 128, nch, 512).transpose(2, 0, 1, 3))

    def tile_seg(seg):
        segT = seg.reshape(HS, D_CTX).T  # [D_CTX, HS]
        a = c(segT[:, :NCH * CHW].reshape(KT_IN, 128, NCH, CHW)
              .transpose(2, 0, 1, 3))
        b = c(segT[:, NCH * CHW:].reshape(KT_IN, 128, TAIL))
        return a, b

    sg1a, sg1b = tile_seg(seg1)
    sg2a, sg2b = tile_seg(seg2)
    shared = {
        "w1t": tile_wt(w1, KT_IN, H // 512),
        "w2t": tile_wt(w2, KT_H, H // 512),
        "wot": tile_wt(w_out, KT_H, D_OUT // 512),
        "sg1a": sg1a, "sg1b": sg1b, "sg2a": sg2a, "sg2b": sg2b,
        "b1d": c(b1.reshape(1, H)), "b2d": c(b2.reshape(1, H)),
        "bod": c(b_out.reshape(1, D_OUT)),
    }
    in_maps = []
    for core in range(NCORES):
        sl = slice(core * BL, (core + 1) * BL)
        m = dict(shared)
        m["xT"] = c(x[sl].T)
        m["ctxT"] = c(context[sl].T)
        in_maps.append(m)
    return in_maps


_NC = None


def kernel(**inputs):
    global _NC
    if _NC is None:
        _NC = build_kernel()
    inputs = {k: np.ascontiguousarray(np.asarray(v), dtype=np.float32)
              for k, v in inputs.items()}
    in_maps = _prep_inputs(**inputs)
    res = run_bass_kernel_spmd(_NC, in_maps, list(range(NCORES)))
    return np.concatenate([res.results[i]["out"] for i in range(NCORES)],
                          axis=0)


# revision 7
# speedup vs baseline: 6741.8380x; 6741.8380x over previous
"""DendriticMLP Trainium2 kernel — 8-core batch-data-parallel, exact fp32.

Architecture (per core, B_local=512 rows):
  y1 = x @ w1.T + b1                (PE, fp32 exact)
  dend1 = ctx @ seg1_flat.T         (PE) -> per-(b,h) max/min over 10 segments (DVE
          strided reduce from PSUM) -> sel = absmax-signed value via
          where(max+min>0, max, min) -> gate = sigmoid(sel) (ACT)
  g = y1 * gate; top-k (k=102) per row via threshold bisection on
          count(g >= t) (DVE tensor_scalar + accum), 23 iterations;
          h = (g >= lo) * g
  h transposed 128x128 blocks on PE for the next layer's stationary operand.
  layer 2 same; out = h2 @ w_out.T + b_out.

All matmuls native fp32 (hi/lo 2-pass in HW, exact); top-k threshold
bisection converges to kth-largest exactly (see where() analysis).
"""
import numpy as np
from contextlib import ExitStack

import concourse.bass as bass
import concourse.tile as tile
from concourse import bacc, mybir, masks
from concourse.bass_utils import run_bass_kernel_spmd

F32 = mybir.dt.float32
AF = mybir.ActivationFunctionType
OP = mybir.AluOpType
AX = mybir.AxisListType

# problem dims (hardcoded per contract)
B, D_IN, H, S, D_CTX, D_OUT = 4096, 1024, 2048, 10, 1024, 1024
KW = 102                 # k-winners per row
NCORES = 8
BL = B // NCORES         # 512 rows per core
BT = BL // 128           # 4 b-tiles of 128 rows
NITER = 23               # bisection iterations

HS = H * S               # 20480
CHW = 510                # dend chunk width (51 groups of 10)
NCH = HS // CHW          # 40 full chunks
TAIL = HS - NCH * CHW    # 80 (8 groups)
GR = CHW // S            # 51 groups per chunk
KT_IN = D_IN // 128      # 8 contraction tiles for d=1024
KT_H = H // 128          # 16 contraction tiles for d=2048


def build_kernel(loop_n=None):
    nc = bacc.Bacc("TRN2", target_bir_lowering=False, debug=False,
                   num_devices=NCORES)

    def din(name, shape):
        return nc.dram_tensor(name, shape, F32, kind="ExternalInput").ap()

    xT = din("xT", [D_IN, BL])
    ctxT = din("ctxT", [D_IN, BL])
    w1t = din("w1t", [H // 512, KT_IN, 128, 512])
    w2t = din("w2t", [H // 512, KT_H, 128, 512])
    wot = din("wot", [D_OUT // 512, KT_H, 128, 512])
    sg1a = din("sg1a", [NCH, KT_IN, 128, CHW])
    sg1b = din("sg1b", [KT_IN, 128, TAIL])
    sg2a = din("sg2a", [NCH, KT_IN, 128, CHW])
    sg2b = din("sg2b", [KT_IN, 128, TAIL])
    b1d = din("b1d", [1, H])
    b2d = din("b2d", [1, H])
    bod = din("bod", [1, D_OUT])
    out_d = nc.dram_tensor("out", [BL, D_OUT], F32, kind="ExternalOutput").ap()

    with tile.TileContext(nc) as tc, ExitStack() as ctx:
        if loop_n is not None:
            ctx.enter_context(tc.For_i(0, loop_n, 1))
        cpool = ctx.enter_context(tc.tile_pool(name="const", bufs=1))
        apool = ctx.enter_context(tc.tile_pool(name="acts", bufs=1))
        ypool = ctx.enter_context(tc.tile_pool(name="y", bufs=BT))
        selpool = ctx.enter_context(tc.tile_pool(name="sel", bufs=BT))
        mnpool = ctx.enter_context(tc.tile_pool(name="mn", bufs=BT))
        htpool = ctx.enter_context(tc.tile_pool(name="ht", bufs=1))
        wpool = ctx.enter_context(tc.tile_pool(name="w", bufs=3))
        spool = ctx.enter_context(tc.tile_pool(name="seg", bufs=8))
        outpool = ctx.enter_context(tc.tile_pool(name="osb", bufs=2))
        tinypool = ctx.enter_context(tc.tile_pool(name="tiny", bufs=1))
        psy = ctx.enter_context(tc.tile_pool(name="psy", bufs=BT, space="PSUM"))
        psd = ctx.enter_context(tc.tile_pool(name="psd", bufs=3, space="PSUM"))
        pst = ctx.enter_context(tc.tile_pool(name="pst", bufs=1, space="PSUM"))

        # constants
        identity = cpool.tile([128, 128], F32)
        masks.make_identity(nc, identity[:])
        ones = cpool.tile([1, 128], F32)
        nc.gpsimd.memset(ones[:], 1.0)
        b1sb = cpool.tile([1, H], F32)
        nc.sync.dma_start(b1sb[:], b1d)
        b2sb = cpool.tile([1, H], F32)
        nc.sync.dma_start(b2sb[:], b2d)
        bosb = cpool.tile([1, D_OUT], F32)
        nc.sync.dma_start(bosb[:], bod)

        # activations stationary: [128, kk*BL + bt*128] layout
        xT_sb = apool.tile([128, KT_IN * BL], F32, tag="xt_scr")
        ctxT_sb = apool.tile([128, KT_IN * BL], F32)
        for kk in range(KT_IN):
            nc.sync.dma_start(xT_sb[:, kk * BL:(kk + 1) * BL],
                              xT[kk * 128:(kk + 1) * 128, :])
            nc.sync.dma_start(ctxT_sb[:, kk * BL:(kk + 1) * BL],
                              ctxT[kk * 128:(kk + 1) * 128, :])

        def st_ap(sb, kk, bt):
            return sb[:, kk * BL + bt * 128: kk * BL + (bt + 1) * 128]

        # per-bt tiny state: cols 0=M 1=lo 2=w 3=t 4=pred 5=cnt
        tiny = [tinypool.tile([128, 8], F32, tag=f"tiny{bt}", name=f"tiny{bt}") for bt in range(BT)]

        h1T = htpool.tile([128, KT_H * 512], F32)   # reused for h2T
        scr_cell = []

        def get_scr():
            if not scr_cell:
                scr_cell.append(apool.tile([128, H], F32, tag="xt_scr",
                                           name="scr"))
            return scr_cell[0]

        def yphase(stat_sb_or_ht, kt, wt_dram, bias_sb, width, lay):
            """Dense y = act @ W.T (+bias). Returns list of BT y tiles [128, width]."""
            ytiles = [ypool.tile([128, H], F32, tag="y", name=f"y{lay}_{i}") for i in range(BT)]
            nch = width // 512
            for n in range(nch):
                ps = [psy.tile([128, 512], F32, tag="psy", name=f"psy{i}") for i in range(BT)]
                for k in range(kt):
                    wt = wpool.tile([128, 512], F32, tag="w")
                    nc.sync.dma_start(wt[:], wt_dram[n, k])
                    for bt in range(BT):
                        if lay == 0:
                            lhsT = st_ap(xT_sb, k, bt)
                        else:
                            lhsT = stat_sb_or_ht[:, k * 512 + bt * 128:
                                                 k * 512 + (bt + 1) * 128]
                        nc.tensor.matmul(ps[bt][:], lhsT, wt[:],
                                         start=(k == 0), stop=False)
                for bt in range(BT):
                    nc.tensor.matmul(ps[bt][:], ones[:],
                                     bias_sb[:, n * 512:(n + 1) * 512],
                                     start=False, stop=True)
                for bt in range(BT):
                    nc.scalar.activation(ytiles[bt][:, n * 512:(n + 1) * 512],
                                         ps[bt][:], AF.Copy)
            return ytiles

        def dendphase(sga, sgb, seltiles, mntiles):
            """dend matmuls + segment max/min reduces into sel (max) and mn."""
            for c in range(NCH + 1):
                w = CHW if c < NCH else TAIL
                g = GR if c < NCH else TAIL // S
                segs = []
                for k in range(KT_IN):
                    stile = spool.tile([128, CHW], F32, tag="seg")
                    if c < NCH:
                        nc.sync.dma_start(stile[:, :w], sga[c, k])
                    else:
                        nc.sync.dma_start(stile[:, :w], sgb[k])
                    segs.append(stile)
                for bt in range(BT):
                    pd = psd.tile([128, CHW], F32, tag="psd")
                    for k in range(KT_IN):
                        nc.tensor.matmul(pd[:, :w], st_ap(ctxT_sb, k, bt),
                                         segs[k][:, :w],
                                         start=(k == 0), stop=(k == KT_IN - 1))
                    view = pd[:, :w].rearrange("p (g s) -> p g s", s=S)
                    nc.vector.tensor_reduce(
                        seltiles[bt][:, c * GR:c * GR + g], view,
                        axis=AX.X, op=OP.max)
                    nc.vector.tensor_reduce(
                        mntiles[bt][:, c * GR:c * GR + g], view,
                        axis=AX.X, op=OP.min)

        def selgate(seltiles, mntiles):
            """sel=where(mx+mn>0,mx,mn) in-place over mx, then sigmoid -> gate."""
            for bt in range(BT):
                mx = seltiles[bt][:]
                mn = mntiles[bt][:]
                scr = get_scr()
                nc.vector.tensor_add(scr[:], mx, mn)
                nc.vector.tensor_scalar(scr[:], scr[:], 0.0, None, op0=OP.is_le)
                nc.vector.copy_predicated(mx, scr[:].bitcast(mybir.dt.int32), mn)
                nc.scalar.activation(mx, mx, AF.Sigmoid)

        def gate_mul(ytiles, seltiles):
            """g = y*gate in place on y tile; M = absmax(g) into tiny col 0."""
            for bt in range(BT):
                nc.vector.tensor_mul(ytiles[bt][:], ytiles[bt][:],
                                     seltiles[bt][:])
                nc.vector.tensor_reduce(tiny[bt][:, 0:1], ytiles[bt][:],
                                        axis=AX.X, op=OP.max,
                                        apply_absolute_value=True)

        def bisect_mask(ytiles):
            """top-k threshold per row via bisection; mask y in place."""
            for bt in range(BT):
                t = tiny[bt]
                M, lo, w, tt_, pred, cnt = (t[:, i:i + 1] for i in range(6))
                nc.vector.tensor_scalar(tt_, M, 1.001, 1e-30, op0=OP.mult,
                                        op1=OP.add)
                nc.vector.tensor_scalar_mul(lo, tt_, -1.0)
                nc.vector.tensor_scalar_mul(w, tt_, 2.0)
            for it in range(NITER):
                for bt in range(BT):
                    t = tiny[bt]
                    M, lo, w, tt_, pred, cnt = (t[:, i:i + 1] for i in range(6))
                    nc.vector.tensor_scalar_mul(w, w, 0.5)
                    nc.vector.tensor_add(tt_, lo, w)
                    nc.vector.scalar_tensor_tensor(
                        get_scr()[:], ytiles[bt][:], tt_, ytiles[bt][:],
                        op0=OP.is_ge, op1=OP.bypass, accum_out=cnt)
                    nc.vector.tensor_scalar(pred, cnt, float(KW), None,
                                            op0=OP.is_ge)
                    nc.vector.scalar_tensor_tensor(lo, pred, w, lo,
                                                   op0=OP.mult, op1=OP.add)
            for bt in range(BT):
                lo = tiny[bt][:, 1:2]
                nc.vector.scalar_tensor_tensor(ytiles[bt][:], ytiles[bt][:],
                                               lo, ytiles[bt][:],
                                               op0=OP.is_ge, op1=OP.mult)

        def transpose_to(ytiles, dst):
            for bt in range(BT):
                for kb in range(KT_H):
                    pt = pst.tile([128, 128], F32, tag="pst")
                    nc.tensor.transpose(pt[:],
                                        ytiles[bt][:, kb * 128:(kb + 1) * 128],
                                        identity[:])
                    nc.scalar.activation(
                        dst[:, kb * 512 + bt * 128: kb * 512 + (bt + 1) * 128],
                        pt[:], AF.Copy)

        # ---------------- layer 1 ----------------
        sel1 = [selpool.tile([128, H], F32, tag="sel", name=f"sel1_{i}") for i in range(BT)]
        mn1 = [mnpool.tile([128, H], F32, tag="mn", name=f"mn1_{i}") for i in range(BT)]
        y1 = yphase(None, KT_IN, w1t, b1sb, H, lay=0)
        dendphase(sg1a, sg1b, sel1, mn1)
        selgate(sel1, mn1)
        gate_mul(y1, sel1)
        bisect_mask(y1)

        # dend2 early (keeps PE busy during layer-1 bisection)
        sel2 = [selpool.tile([128, H], F32, tag="sel", name=f"sel2_{i}") for i in range(BT)]
        mn2 = [mnpool.tile([128, H], F32, tag="mn", name=f"mn2_{i}") for i in range(BT)]
        dendphase(sg2a, sg2b, sel2, mn2)
        selgate(sel2, mn2)

        transpose_to(y1, h1T)

        # ---------------- layer 2 ----------------
        y2 = yphase(h1T[:], KT_H, w2t, b2sb, H, lay=1)
        gate_mul(y2, sel2)
        bisect_mask(y2)
        transpose_to(y2, h1T)  # h2T reuses the same arena

        # ---------------- output layer ----------------
        for n in range(D_OUT // 512):
            ps = [psy.tile([128, 512], F32, tag="psy", name=f"psy{i}") for i in range(BT)]
            for k in range(KT_H):
                wt = wpool.tile([128, 512], F32, tag="w")
                nc.sync.dma_start(wt[:], wot[n, k])
                for bt in range(BT):
                    lhsT = h1T[:, k * 512 + bt * 128: k * 512 + (bt + 1) * 128]
                    nc.tensor.matmul(ps[bt][:], lhsT, wt[:],
                                     start=(k == 0), stop=False)
            for bt in range(BT):
                nc.tensor.matmul(ps[bt][:], ones[:],
                                 bosb[:, n * 512:(n + 1) * 512],
                                 start=False, stop=True)
            for bt in range(BT):
                osb = outpool.tile([128, 512], F32, tag="osb")
                nc.scalar.activation(osb[:], ps[bt][:], AF.Copy)
                nc.sync.dma_start(
                    out_d[bt * 128:(bt + 1) * 128, n * 512:(n + 1) * 512],
                    osb[:])

    nc.compile()
    return nc


def _prep_inputs(x, context, w1, b1, seg1, w2, b2, seg2, w_out, b_out):
    """Host-side reshapes into the DMA-friendly tiled layouts."""
    c = np.ascontiguousarray

    def tile_wt(w, kt, nch):
        # w [out, in] -> wT [in, out] -> [nch, kt, 128, 512]
        wT = w.T
        return c(wT.reshape(kt, 128, nch, 512).transpose(2, 0, 1, 3))

    def tile_seg(seg):
        segT = seg.reshape(HS, D_CTX).T  # [D_CTX, HS]
        a = c(segT[:, :NCH * CHW].reshape(KT_IN, 128, NCH, CHW)
              .transpose(2, 0, 1, 3))
        b = c(segT[:, NCH * CHW:].reshape(KT_IN, 128, TAIL))
        return a, b

    sg1a, sg1b = tile_seg(seg1)
    sg2a, sg2b = tile_seg(seg2)
    shared = {
        "w1t": tile_wt(w1, KT_IN, H // 512),
        "w2t": tile_wt(w2, KT_H, H // 512),
        "wot": tile_wt(w_out, KT_H, D_OUT // 512),
        "sg1a": sg1a, "sg1b": sg1b, "sg2a": sg2a, "sg2b": sg2b,
        "b1d": c(b1.reshape(1, H)), "b2d": c(b2.reshape(1, H)),
        "bod": c(b_out.reshape(1, D_OUT)),
    }
    in_maps = []
    for core in range(NCORES):
        sl = slice(core * BL, (core + 1) * BL)
        m = dict(shared)
        m["xT"] = c(x[sl].T)
        m["ctxT"] = c(context[sl].T)
        in_maps.append(m)
    return in_maps


_NC = None


def kernel(**inputs):
    global _NC
    if _NC is None:
        _NC = build_kernel()
    inputs = {k: np.ascontiguousarray(np.asarray(v), dtype=np.float32)
              for k, v in inputs.items()}
    in_maps = _prep_inputs(**inputs)
    res = run_bass_kernel_spmd(_NC, in_maps, list(range(NCORES)))
    return np.concatenate([res.results[i]["out"] for i in range(NCORES)],
                          axis=0)
